# revision 1
# baseline (speedup 1.0000x reference)
"""Trainium2 Bass kernel: conv q/k/v -> per-channel row attention -> output conv.

Sharding: data-parallel over batch B=8, one batch element per NeuronCore.
Compute dtype: bf16 matmul inputs, fp32 PSUM accumulation.

Per-core layout plan:
  - convs run channel-on-partition (orientation A): psum[co=128, f=384] accumulating
    18 matmuls (2 ci blocks x 9 taps), weights stationary.
  - attention runs spatial-on-partition per channel: q_t/k_t [w, c, i],
    attnT = K_c^T.T @ Q_c^T -> psum [g? actually [g,i]] per channel; exp on ACT;
    mm2: fused[i, w'] = expT_c.T @ V'_c with a ones column appended to V for the
    softmax denominator (no max subtraction needed: logits are O(1)).
  - layout changes between conv-space and attention-space are PE transposes
    (identity matmuls) interleaved into the conv epilogues.
"""

import numpy as np
import ml_dtypes

B, C, H, W, K = 8, 256, 96, 96, 3
HP, WP = H + 2, W + 2          # padded spatial
S = H * W                      # 9216
FT_ROWS = 4                    # rows (or cols) per conv f-tile
FT = FT_ROWS * W               # 384 free per conv matmul
NFT = H // FT_ROWS             # 24 f-tiles
NCB = C // 128                 # 2 channel blocks
GRP = 5                        # attention channels per psum bank group

bf16 = ml_dtypes.bfloat16

_cache = {}


def _pack_weights(w):
    # w [co, ci, kh, kw] fp32 -> [ci'=128, cib, off, cob, co'=128] bf16
    w = np.asarray(w, np.float32)
    wt = w.transpose(1, 2, 3, 0)                     # [ci, kh, kw, co]
    wt = wt.reshape(NCB, 128, K * K, NCB, 128)       # [cib, ci', off, cob, co']
    wt = wt.transpose(1, 0, 2, 3, 4)                 # [ci', cib, off, cob, co']
    return np.ascontiguousarray(wt).astype(bf16)


def _pack_bias(b):
    return np.asarray(b, np.float32).reshape(NCB, 128).T.copy()  # [co'=128, cob]


def build_nc(wq, bq, wk, bk, wv, bv, wo, bo):
    import concourse.mybir as mybir
    import concourse.tile as tile
    from concourse import bacc

    dt = mybir.dt
    AF = mybir.ActivationFunctionType

    scale = np.float32(1.0 / np.sqrt(np.float32(W)))
    wpacks = [_pack_weights(wq), _pack_weights(np.asarray(wk) * scale),
              _pack_weights(wv), _pack_weights(wo)]
    bpack = np.stack([_pack_bias(bq), _pack_bias(np.asarray(bk) * scale),
                      _pack_bias(bv), _pack_bias(bo)], axis=1)  # [128, 4, 2]

    nc = bacc.Bacc(None, target_bir_lowering=False)
    gp = nc.dram_tensor("gp", [C, HP * WP], dt.bfloat16, kind="ExternalInput")
    xp = nc.dram_tensor("xp", [C, HP * WP], dt.bfloat16, kind="ExternalInput")
    out = nc.dram_tensor("out", [C, S], dt.float32, kind="ExternalOutput")

    w_dram = [nc.inline_tensor(wpacks[i], name=f"w{i}") for i in range(4)]
    b_dram = nc.inline_tensor(bpack, name="bias")
    id_dram = nc.inline_tensor(np.eye(128).astype(bf16), name="ident")

    with tile.TileContext(nc) as tc:
        with tc.tile_pool(name="persist", bufs=1) as pper, \
             tc.tile_pool(name="big", bufs=1) as pbig, \
             tc.tile_pool(name="w", bufs=2) as pw, \
             tc.tile_pool(name="win", bufs=6) as pwin, \
             tc.tile_pool(name="outst", bufs=3) as pout, \
             tc.tile_pool(name="rc", bufs=2) as prc, \
             tc.tile_pool(name="ps", bufs=1, space="PSUM") as pp:

            ident = pper.tile([128, 128], dt.bfloat16, tag="ident")
            nc.sync.dma_start(ident[:], id_dram[:])
            b_sb = pper.tile([128, 4, 2], dt.float32, tag="bias")
            nc.sync.dma_start(b_sb[:], b_dram[:])

            def load_w(i):
                w_sb = pw.tile([128, NCB, K * K, NCB, 128], dt.bfloat16, tag="w")
                nc.sync.dma_start(w_sb[:], w_dram[i][:])
                return w_sb

            def load_pad(dram, region):
                t = pbig.tile([128, NCB, HP, WP], dt.bfloat16, tag=region)
                for cib in range(NCB):
                    nc.sync.dma_start(
                        t[:, cib],
                        dram[cib * 128:(cib + 1) * 128, :].rearrange(
                            "p (h w) -> p h w", w=WP))
                return t

            def conv(src, w_sb, conv_idx, col_major, epilogue):
                """src: [128, NCB, HP, WP] padded input; epilogue(cob, ft, win)."""
                for cob in range(NCB):
                    for ft in range(NFT):
                        ps = pp.tile([128, FT], dt.float32, tag="conv")
                        n = 0
                        for cib in range(NCB):
                            for di in range(K):
                                for dj in range(K):
                                    lhsT = w_sb[:, cib, di * K + dj, cob, :]
                                    if col_major:
                                        rhs = src[:, cib, di:di + H,
                                                  ft * FT_ROWS + dj:
                                                  ft * FT_ROWS + dj + FT_ROWS]
                                        rhs = rhs.rearrange("p r c -> p c r")
                                    else:
                                        rhs = src[:, cib,
                                                  ft * FT_ROWS + di:
                                                  ft * FT_ROWS + di + FT_ROWS,
                                                  dj:dj + W]
                                    nc.tensor.matmul(ps[:], lhsT, rhs,
                                                     start=(n == 0),
                                                     stop=(n == 2 * K * K - 1))
                                    n += 1
                        win = pwin.tile([128, FT_ROWS, W], dt.bfloat16, tag="win")
                        nc.scalar.activation(
                            win[:].rearrange("p r c -> p (r c)"), ps[:],
                            AF.Identity, bias=b_sb[:, conv_idx, cob:cob + 1])
                        epilogue(cob, ft, win)

            def epi_to_att(dst):
                # row-major conv out window [c,(r,w)] -> dst[w_or_i, c, row]
                def _e(cob, ft, win):
                    for r in range(FT_ROWS):
                        tp = pp.tile([128, 128], dt.bfloat16, tag="tp")
                        nc.tensor.transpose(tp[:96, :128], win[:, r, :], ident[:])
                        nc.vector.tensor_copy(
                            dst[:, cob * 128:(cob + 1) * 128, ft * FT_ROWS + r],
                            tp[:96, :128])
                return _e

            # ---------------- phase 1: conv q ----------------
            g_sb = load_pad(gp, "R1")
            x_sb1 = load_pad(xp, "R2")
            wq_sb = load_w(0)
            q_t = pbig.tile([96, C, W], dt.bfloat16, tag="R3")
            conv(g_sb, wq_sb, 0, False, epi_to_att(q_t))

            # ---------------- phase 2: conv k ----------------
            wk_sb = load_w(1)
            k_t = pbig.tile([96, C, W], dt.bfloat16, tag="R1")
            conv(x_sb1, wk_sb, 1, False, epi_to_att(k_t))

            # ---------------- phase 3: attn logits + exp ----------------
            expT = pbig.tile([96, C, W], dt.bfloat16, tag="R2")
            for c0 in range(0, C, GRP):
                gsz = min(GRP, C - c0)
                ps = pp.tile([128, GRP * (W + 1)], dt.float32, tag="att")
                for j in range(gsz):
                    c = c0 + j
                    nc.tensor.matmul(ps[:96, j * W:(j + 1) * W],
                                     k_t[:, c, :], q_t[:, c, :],
                                     start=(j == 0), stop=(j == gsz - 1))
                ps3 = ps[:96, :gsz * W].rearrange("p (c w) -> p c w", w=W)
                nc.scalar.activation(expT[:, c0:c0 + gsz, :], ps3, AF.Exp)

            # ---------------- phase 4: conv v (col-major) ----------------
            x_sb2 = load_pad(xp, "R3")
            wv_sb = load_w(2)
            v_r = pbig.tile([96, C, W + 1], dt.bfloat16, tag="R1")
            nc.vector.memset(v_r[:96, :, W], 1.0)

            def epi_v(cob, ft, win):
                # col-major window [c, (wl, i)] -> v_r[g=i, c, w0+wl]
                for wl in range(FT_ROWS):
                    tp = pp.tile([128, 128], dt.bfloat16, tag="tp")
                    nc.tensor.transpose(tp[:96, :128], win[:, wl, :], ident[:])
                    nc.vector.tensor_copy(
                        v_r[:, cob * 128:(cob + 1) * 128, ft * FT_ROWS + wl],
                        tp[:96, :128])

            conv(x_sb2, wv_sb, 2, True, epi_v)

            # ---------------- phase 5: mm2 + normalize ----------------
            fused_t = pbig.tile([96, C, W], dt.bfloat16, tag="R3")
            for c0 in range(0, C, GRP):
                gsz = min(GRP, C - c0)
                ps = pp.tile([128, GRP * (W + 1)], dt.float32, tag="att")
                for j in range(gsz):
                    c = c0 + j
                    nc.tensor.matmul(ps[:96, j * (W + 1):(j + 1) * (W + 1)],
                                     expT[:, c, :], v_r[:, c, :],
                                     start=(j == 0), stop=(j == gsz - 1))
                ps3 = ps[:96, :gsz * (W + 1)].rearrange(
                    "p (c w) -> p c w", w=W + 1)
                rc = prc.tile([96, GRP], dt.float32, tag="rc")
                nc.vector.reciprocal(rc[:96, :gsz], ps3[:, :, W])
                nc.vector.tensor_tensor(
                    fused_t[:, c0:c0 + gsz, :], ps3[:, :, :W],
                    rc[:96, :gsz, None].to_broadcast((96, gsz, W)),
                    mybir.AluOpType.mult)

            # ---------------- phase 6: fused_t -> fused_pad ----------------
            f_pad = pbig.tile([128, NCB, HP, WP], dt.bfloat16, tag="R2")
            for cib in range(NCB):
                nc.vector.memset(f_pad[:, cib, 0, :], 0.0)
                nc.vector.memset(f_pad[:, cib, HP - 1, :], 0.0)
                nc.vector.memset(f_pad[:, cib, 1:HP - 1, 0], 0.0)
                nc.vector.memset(f_pad[:, cib, 1:HP - 1, WP - 1], 0.0)
            for cib in range(NCB):
                for wp_ in range(W):
                    tp = pp.tile([128, 128], dt.bfloat16, tag="tp")
                    nc.tensor.transpose(
                        tp[:128, :96],
                        fused_t[:, cib * 128:(cib + 1) * 128, wp_],
                        ident[:96, :96])
                    nc.vector.tensor_copy(
                        f_pad[:, cib, 1:HP - 1, wp_ + 1], tp[:128, :96])

            # ---------------- phase 7: conv o -> out ----------------
            wo_sb = load_w(3)

            def epi_o(cob, ft, win):
                pass  # unused; conv-o writes f32 directly below

            for cob in range(NCB):
                for ft in range(NFT):
                    ps = pp.tile([128, FT], dt.float32, tag="conv")
                    n = 0
                    for cib in range(NCB):
                        for di in range(K):
                            for dj in range(K):
                                lhsT = wo_sb[:, cib, di * K + dj, cob, :]
                                rhs = f_pad[:, cib,
                                            ft * FT_ROWS + di:
                                            ft * FT_ROWS + di + FT_ROWS,
                                            dj:dj + W]
                                nc.tensor.matmul(ps[:], lhsT, rhs,
                                                 start=(n == 0),
                                                 stop=(n == 2 * K * K - 1))
                                n += 1
                    ost = pout.tile([128, FT], dt.float32, tag="outst")
                    nc.scalar.activation(ost[:], ps[:], AF.Identity,
                                         bias=b_sb[:, 3, cob:cob + 1])
                    nc.sync.dma_start(
                        out[cob * 128:(cob + 1) * 128, ft * FT:(ft + 1) * FT],
                        ost[:])

    nc.finalize()
    return nc


def _pad_cast(x):
    # x [C, H, W] fp32 -> [C, HP*WP] bf16 zero-padded
    xp = np.zeros((C, HP, WP), np.float32)
    xp[:, 1:H + 1, 1:W + 1] = x
    return xp.reshape(C, HP * WP).astype(bf16)


def kernel(feats, graph_feature, wq, bq, wk, bk, wv, bv, wo, bo):
    from concourse.bass_utils import run_bass_kernel_spmd

    key = "nc"
    if key not in _cache:
        _cache[key] = build_nc(wq, bq, wk, bk, wv, bv, wo, bo)
    nc = _cache[key]

    in_maps = []
    for b in range(B):
        in_maps.append({
            "gp": _pad_cast(np.asarray(graph_feature[b], np.float32)),
            "xp": _pad_cast(np.asarray(feats[b], np.float32)),
        })
    res = run_bass_kernel_spmd(nc, in_maps, core_ids=list(range(B)))
    outs = [res.results[b]["out"].reshape(C, H, W) for b in range(B)]
    return np.stack(outs).astype(np.float32)


# revision 8
# speedup vs baseline: 1.4201x; 1.4201x over previous
"""Trainium2 Bass kernel: conv q/k/v -> per-channel row attention -> output conv.

Sharding: data-parallel over batch B=8, one batch element per NeuronCore.
Compute dtype: bf16 matmul inputs, fp32 PSUM accumulation.

Per-core plan:
  - convs run channel-on-partition: psum[co=128, f=384] accumulates 18 matmuls
    (2 ci blocks x 9 taps) with weight tiles stationary.
  - conv-v reads a host-pre-transposed padded image so its f-tiles are
    column-panels with contiguous reads; its epilogue transposes land directly
    in attention layout v_r[g, c, w'] (with a ones column for the softmax
    denominator; no max-subtraction needed, logits are O(1)).
  - attention is per-channel: attnT psum [g,i] = K_c^T.T @ Q_c^T; exp on ACT;
    fused[i,w'] = expT_c.T @ V'_c; denominator rides in column W of V'.
  - PE transposes (identity matmuls) move conv outputs into attention layout;
    8 transposes share one PSUM bank (start only on slot 0) so one DVE copy
    with 8-element contiguous runs drains each bank.
"""

import numpy as np
import ml_dtypes

B, C, H, W, K = 8, 256, 96, 96, 3
HP, WP = H + 2, W + 2
S = H * W
FT_ROWS = 4
FT = FT_ROWS * W               # 384
NFT = H // FT_ROWS             # 24
NCB = C // 128
GRP = 5

bf16 = ml_dtypes.bfloat16

_cache = {}


def _pack_weights(w):
    w = np.asarray(w, np.float32)
    wt = w.transpose(1, 2, 3, 0)                     # [ci, kh, kw, co]
    wt = wt.reshape(NCB, 128, K * K, NCB, 128)       # [cib, ci', off, cob, co']
    wt = wt.transpose(1, 0, 2, 3, 4)                 # [ci', cib, off, cob, co']
    return np.ascontiguousarray(wt).astype(bf16)


def _pack_bias(b):
    return np.asarray(b, np.float32).reshape(NCB, 128).T.copy()


def build_nc(wq, bq, wk, bk, wv, bv, wo, bo):
    import concourse.mybir as mybir
    import concourse.tile as tile
    from concourse import bacc

    dt = mybir.dt
    AF = mybir.ActivationFunctionType

    scale = np.float32(1.0 / np.sqrt(np.float32(W)))
    wpacks = [_pack_weights(wq), _pack_weights(np.asarray(wk) * scale),
              _pack_weights(wv), _pack_weights(wo)]
    bpack = np.stack([_pack_bias(bq), _pack_bias(np.asarray(bk) * scale),
                      _pack_bias(bv), _pack_bias(bo)], axis=1)  # [128, 4, 2]

    nc = bacc.Bacc(None, target_bir_lowering=False)
    gp = nc.dram_tensor("gp", [C, HP * WP], dt.bfloat16, kind="ExternalInput")
    xp = nc.dram_tensor("xp", [C, HP * WP], dt.bfloat16, kind="ExternalInput")
    xpt = nc.dram_tensor("xpt", [C, HP * WP], dt.bfloat16, kind="ExternalInput")
    out = nc.dram_tensor("out", [C, S], dt.float32, kind="ExternalOutput")

    w_dram = [nc.inline_tensor(wpacks[i], name=f"w{i}") for i in range(4)]
    b_dram = nc.inline_tensor(bpack, name="bias")
    id_dram = nc.inline_tensor(np.eye(128).astype(bf16), name="ident")

    with tile.TileContext(nc) as tc:
        with tc.tile_pool(name="persist", bufs=1) as pper, \
             tc.tile_pool(name="big", bufs=1) as pbig, \
             tc.tile_pool(name="w", bufs=2) as pw, \
             tc.tile_pool(name="win", bufs=6) as pwin, \
             tc.tile_pool(name="outst", bufs=3) as pout, \
             tc.tile_pool(name="rc", bufs=2) as prc, \
             tc.tile_pool(name="ps", bufs=1, space="PSUM") as pp:

            ident = pper.tile([128, 128], dt.bfloat16, tag="ident")
            nc.sync.dma_start(ident[:], id_dram[:])
            b_sb = pper.tile([128, 4, 2], dt.float32, tag="bias")
            nc.sync.dma_start(b_sb[:], b_dram[:])

            def load_w(i):
                w_sb = pw.tile([128, NCB, K * K, NCB, 128], dt.bfloat16, tag="w")
                nc.sync.dma_start(w_sb[:], w_dram[i][:])
                return w_sb

            def load_pad(dram, region):
                t = pbig.tile([128, NCB, HP, WP], dt.bfloat16, tag=region)
                for cib in range(NCB):
                    nc.sync.dma_start(
                        t[:, cib],
                        dram[cib * 128:(cib + 1) * 128, :].rearrange(
                            "p (h w) -> p h w", w=WP))
                return t

            ROW_TILES = [(r, min(5, H - r)) for r in range(0, H, 5)]

            def conv(src, w_sb, conv_idx, swap_taps, epilogue):
                """src: [128, NCB, HP, WP]; epilogue(cob, row0, nrows, win);
                swap_taps: src is the transposed image (taps di/dj swapped)."""
                for cob in range(NCB):
                    for row0, nrows in ROW_TILES:
                        ps = pp.tile([128, 5 * W], dt.float32, tag="conv",
                                     name="psc", bufs=3)
                        n = 0
                        for cib in range(NCB):
                            for di in range(K):
                                for dj in range(K):
                                    lhsT = w_sb[:, cib, di * K + dj, cob, :]
                                    r0, c0 = (dj, di) if swap_taps else (di, dj)
                                    rhs = src[:, cib, row0 + r0:row0 + r0 + nrows,
                                              c0:c0 + W]
                                    nc.tensor.matmul(ps[:, :nrows * W]
                                                     .rearrange("p (r c) -> p r c", c=W),
                                                     lhsT, rhs,
                                                     start=(n == 0),
                                                     stop=(n == 2 * K * K - 1))
                                    n += 1
                        win = pwin.tile([128, 5, W], dt.bfloat16, tag="win",
                                        name="winc")
                        nc.scalar.activation(
                            win[:, :nrows].rearrange("p r c -> p (r c)"),
                            ps[:, :nrows * W],
                            AF.Identity, bias=b_sb[:, conv_idx, cob:cob + 1])
                        epilogue(cob, row0, nrows, win)

            def epi_to_att(dst, last_stride_is_one):
                """Accumulate row-transposes 8 per psum bank, then one DVE
                copy with 8-contiguous runs. dst[w_or_g, c, m]."""
                state = {'tp': None}

                def _e(cob, row0, nrows, win):
                    for r in range(nrows):
                        m = row0 + r
                        slot = m % 8
                        if slot == 0:
                            state['tp'] = pp.tile([128, 8, 128], dt.bfloat16,
                                                  name="tp8", tag="tp", bufs=3)
                        nc.tensor.matmul(state['tp'][:96, slot, :],
                                         win[:, r, :],
                                         ident[:], is_transpose=True,
                                         start=(slot == 0), stop=(slot == 7))
                        if slot == 7:
                            m0 = m - 7
                            nc.vector.tensor_copy(
                                dst[:, cob * 128:(cob + 1) * 128, m0:m0 + 8],
                                state['tp'][:96].rearrange("p r c -> p c r"))
                return _e

            # ---------------- phase 1: conv q ----------------
            g_sb = load_pad(gp, "R1")
            x_sb1 = load_pad(xp, "R2")
            wq_sb = load_w(0)
            q_t = pbig.tile([96, C, W], dt.bfloat16, tag="R3")
            conv(g_sb, wq_sb, 0, False, epi_to_att(q_t, False))

            # ---------------- phase 2: conv k ----------------
            wk_sb = load_w(1)
            k_t = pbig.tile([96, C, W], dt.bfloat16, tag="R1")
            conv(x_sb1, wk_sb, 1, False, epi_to_att(k_t, False))

            # ---------------- phase 3: attn logits + exp ----------------
            expT = pbig.tile([96, C, W], dt.bfloat16, tag="R2")
            for c0 in range(0, C, GRP):
                gsz = min(GRP, C - c0)
                ps = pp.tile([128, GRP * (W + 1)], dt.float32, tag="att", bufs=2)
                for j in range(gsz):
                    c = c0 + j
                    nc.tensor.matmul(ps[:96, j * W:(j + 1) * W],
                                     k_t[:, c, :], q_t[:, c, :],
                                     start=(j == 0), stop=(j == gsz - 1))
                ps3 = ps[:96, :gsz * W].rearrange("p (c w) -> p c w", w=W)
                nc.scalar.activation(expT[:, c0:c0 + gsz, :], ps3, AF.Exp)

            # ---------------- phase 4: conv v (transposed image) ----------
            xt_sb = load_pad(xpt, "R3")
            wv_sb = load_w(2)
            v_r = pbig.tile([96, C, W + 1], dt.bfloat16, tag="R1")
            nc.vector.memset(v_r[:96, :, W], 1.0)
            conv(xt_sb, wv_sb, 2, True, epi_to_att(v_r, True))

            # ---------------- phase 5: mm2 + normalize ----------------
            fused_t = pbig.tile([96, C, W], dt.bfloat16, tag="R3")
            for c0 in range(0, C, GRP):
                gsz = min(GRP, C - c0)
                ps = pp.tile([128, GRP * (W + 1)], dt.float32, tag="att", bufs=2)
                for j in range(gsz):
                    c = c0 + j
                    nc.tensor.matmul(ps[:96, j * (W + 1):(j + 1) * (W + 1)],
                                     expT[:, c, :], v_r[:, c, :],
                                     start=(j == 0), stop=(j == gsz - 1))
                ps3 = ps[:96, :gsz * (W + 1)].rearrange(
                    "p (c w) -> p c w", w=W + 1)
                rc = prc.tile([96, GRP], dt.float32, tag="rc")
                nc.vector.reciprocal(rc[:96, :gsz], ps3[:, :, W])
                nc.vector.tensor_tensor(
                    fused_t[:, c0:c0 + gsz, :], ps3[:, :, :W],
                    rc[:96, :gsz, None].to_broadcast((96, gsz, W)),
                    mybir.AluOpType.mult)

            # ---------------- phase 6: fused_t -> fused_pad ----------------
            f_pad = pbig.tile([128, NCB, HP, WP], dt.bfloat16, tag="R2")
            for cib in range(NCB):
                nc.vector.memset(f_pad[:, cib, 0, :], 0.0)
                nc.vector.memset(f_pad[:, cib, HP - 1, :], 0.0)
                nc.vector.memset(f_pad[:, cib, 1:HP - 1, 0], 0.0)
                nc.vector.memset(f_pad[:, cib, 1:HP - 1, WP - 1], 0.0)
            for cib in range(NCB):
                for w0 in range(0, W, 8):
                    tp = pp.tile([128, 8, 96], dt.bfloat16, tag="tp", bufs=3)
                    for wl in range(8):
                        nc.tensor.matmul(
                            tp[:128, wl, :],
                            fused_t[:, cib * 128:(cib + 1) * 128, w0 + wl],
                            ident[:96, :96], is_transpose=True,
                            start=(wl == 0), stop=(wl == 7))
                    nc.vector.tensor_copy(
                        f_pad[:, cib, 1:HP - 1, w0 + 1:w0 + 9],
                        tp[:128].rearrange("p w i -> p i w"))

            # ---------------- phase 7: conv o -> out ----------------
            wo_sb = load_w(3)

            def epi_o(cob, row0, nrows, win):
                raise AssertionError("unused")

            for cob in range(NCB):
                for row0, nrows in ROW_TILES:
                    ps = pp.tile([128, 5 * W], dt.float32, tag="conv",
                                 name="pso", bufs=3)
                    n = 0
                    for cib in range(NCB):
                        for di in range(K):
                            for dj in range(K):
                                lhsT = wo_sb[:, cib, di * K + dj, cob, :]
                                rhs = f_pad[:, cib, row0 + di:row0 + di + nrows,
                                            dj:dj + W]
                                nc.tensor.matmul(ps[:, :nrows * W]
                                                 .rearrange("p (r c) -> p r c", c=W),
                                                 lhsT, rhs,
                                                 start=(n == 0),
                                                 stop=(n == 2 * K * K - 1))
                                n += 1
                    ost = pout.tile([128, 5 * W], dt.float32, tag="outst",
                                    name="osto")
                    nc.scalar.activation(ost[:, :nrows * W], ps[:, :nrows * W],
                                         AF.Identity,
                                         bias=b_sb[:, 3, cob:cob + 1])
                    nc.sync.dma_start(
                        out[cob * 128:(cob + 1) * 128,
                            row0 * W:(row0 + nrows) * W],
                        ost[:, :nrows * W])

    nc.finalize()
    return nc


def _pad_cast(x):
    xp = np.zeros((C, HP, WP), np.float32)
    xp[:, 1:H + 1, 1:W + 1] = x
    return xp.reshape(C, HP * WP).astype(bf16)


def _pad_cast_t(x):
    xp = np.zeros((C, HP, WP), np.float32)
    xp[:, 1:H + 1, 1:W + 1] = x
    xpt = np.swapaxes(xp, 1, 2)
    return np.ascontiguousarray(xpt).reshape(C, HP * WP).astype(bf16)


def build_sharded(nc):
    """Persistent sharded jit over 8 cores (no donation: reusable buffers)."""
    import jax
    from jax.sharding import Mesh, PartitionSpec
    from jax.experimental.shard_map import shard_map
    import concourse.mybir as mybir
    from concourse import bass2jax

    bass2jax.install_neuronx_cc_hook()
    part_name = nc.partition_id_tensor.name if nc.partition_id_tensor else None
    in_names, out_names, out_avals = [], [], []
    for alloc in nc.m.functions[0].allocations:
        if not isinstance(alloc, mybir.MemoryLocationSet):
            continue
        name = alloc.memorylocations[0].name
        if alloc.kind == 'ExternalInput':
            if name != part_name:
                in_names.append(name)
        elif alloc.kind == 'ExternalOutput':
            out_names.append(name)
            out_avals.append(jax.core.ShapedArray(tuple(alloc.tensor_shape),
                                                  mybir.dt.np(alloc.dtype)))
    all_in = in_names + out_names + ([part_name] if part_name else [])

    def _body(*args):
        ops = list(args)
        if part_name:
            ops.append(bass2jax.partition_id_tensor())
        return tuple(bass2jax._bass_exec_p.bind(
            *ops, out_avals=tuple(out_avals), in_names=tuple(all_in),
            out_names=tuple(out_names), lowering_input_output_aliases=(),
            sim_require_finite=True, sim_require_nnan=True, nc=nc))

    devices = jax.devices()[:B]
    mesh = Mesh(np.asarray(devices), ('core',))
    sharded = jax.jit(shard_map(
        _body, mesh=mesh,
        in_specs=(PartitionSpec('core'),) * (len(in_names) + len(out_names)),
        out_specs=(PartitionSpec('core'),) * len(out_names),
        check_rep=False), keep_unused=True)
    return sharded, in_names, out_names, out_avals


def make_in_maps(feats, graph_feature):
    return [{
        "gp": _pad_cast(np.asarray(graph_feature[b], np.float32)),
        "xp": _pad_cast(np.asarray(feats[b], np.float32)),
        "xpt": _pad_cast_t(np.asarray(feats[b], np.float32)),
    } for b in range(B)]


def kernel(feats, graph_feature, wq, bq, wk, bk, wv, bv, wo, bo):
    import jax

    if "nc" not in _cache:
        _cache["nc"] = build_nc(wq, bq, wk, bk, wv, bv, wo, bo)
        _cache["sharded"] = build_sharded(_cache["nc"])
    sharded, in_names, out_names, out_avals = _cache["sharded"]

    in_maps = make_in_maps(feats, graph_feature)
    concat_in = [np.concatenate([in_maps[c][n] for c in range(B)], axis=0)
                 for n in in_names]
    concat_zeros = [np.zeros((B * a.shape[0], *a.shape[1:]), a.dtype)
                    for a in out_avals]
    r = sharded(*concat_in, *concat_zeros)
    jax.block_until_ready(r)
    o = np.asarray(r[out_names.index("out")]).reshape(B, C, H, W)
    return o.astype(np.float32)
